# revision 1
# baseline (speedup 1.0000x reference)
"""TRN2 Bass/Tile kernel for nn_Loss_58317065945194.

Loss: per-sample EMD with r=2 over C=10 channels:
    d = p - q                       # [B, C]
    S = cumsum(d, axis=1)           # per-sample prefix sums
    per_sample = sqrt(mean(S**2))   # [B]
    out = mean(per_sample)          # scalar

Strategy (pure data parallel, 8 cores):
  - Shard B across 8 cores; per core reshape the [Bs, 10] shard to
    [128 partitions, 20480] (each partition holds 2048 whole samples,
    10 contiguous values each). Inputs are cast to fp16 host-side
    (halves HBM traffic; scan state stays fp32 internally).
  - Per chunk of W samples/partition:
      * Vector:  one tensor_tensor_scan fuses the subtract with the
                 running prefix sum: S[t] = (p[t] + state) - q[t]
                 (state kept fp32 internally; output fp16)
      * GpSimd/Vector (alternating): per-sample prefix sums recovered
                 by subtracting each sample's start boundary
                 (broadcast AP with step 0)
      * Scalar:  sq = c^2  (in place)
      * Vector:  U[g] = sum_j sq[g, j]   (3D AP, reduce axis=X)
      * Scalar:  loss = sqrt(U / C), accum_out -> per-chunk column
  - Each core returns [128, NCHUNK] fp32 partial sums of per-sample
    losses; the host sums all partials and divides by B (replaces the
    all-reduce).
"""

import sys

import numpy as np

if "/opt/trn_rl_repo" not in sys.path:
    sys.path.insert(0, "/opt/trn_rl_repo")

N_CORES = 8
B, C = 2097152, 10
BS = B // N_CORES        # samples per core shard
P = 128                  # SBUF partitions
FPP = BS * C // P        # elems per partition (20480)
W = 256                  # samples per chunk per partition
CW = W * C               # chunk free width (2560)
NCHUNK = FPP // CW       # chunks per core (8)

_cache = {}


def _build_program():
    import concourse.tile as tile
    from concourse import bacc, mybir

    f32, f16 = mybir.dt.float32, mybir.dt.float16
    Alu = mybir.AluOpType
    Act = mybir.ActivationFunctionType

    nc = bacc.Bacc(
        "TRN2", target_bir_lowering=False, debug=False, num_devices=N_CORES
    )
    p_d = nc.dram_tensor("p", [P, FPP], f16, kind="ExternalInput").ap()
    q_d = nc.dram_tensor("q", [P, FPP], f16, kind="ExternalInput").ap()
    o_d = nc.dram_tensor("partial", [P, NCHUNK], f32, kind="ExternalOutput").ap()

    with tile.TileContext(nc) as tc:
        with (
            tc.tile_pool(name="io", bufs=4) as io,
            tc.tile_pool(name="work", bufs=4) as work,
            tc.tile_pool(name="small", bufs=2) as small,
            tc.tile_pool(name="accp", bufs=1) as accp,
        ):
            acc = accp.tile([P, NCHUNK], f32)
            for ci in range(NCHUNK):
                pt = io.tile([P, CW], f16, tag="p")
                qt = io.tile([P, CW], f16, tag="q")
                nc.sync.dma_start(pt[:], p_d[:, ci * CW : (ci + 1) * CW])
                nc.sync.dma_start(qt[:], q_d[:, ci * CW : (ci + 1) * CW])

                # fused subtract + running prefix sum on Vector:
                # S[8+t] = (p[t] + state) - q[t]; S[7] = 0 (memset).
                # Scan output starts at offset 8 (16B) to keep it aligned.
                # S crosses sample boundaries; fixed up below.
                S = work.tile([P, CW + 8], f16, tag="S")
                nc.gpsimd.memset(S[:, 7:8], 0.0)
                nc.vector.tensor_tensor_scan(
                    S[:, 8:], pt[:], qt[:], 0.0, Alu.add, Alu.subtract
                )

                # per-sample prefix sums: c[g, j] = S[8+10g+j] - S[8+10g-1]
                # (broadcast subtract; alternate gpsimd/vector to balance)
                s3 = S[:, 8:].rearrange("p (w c) -> p w c", c=C)
                b3 = S[:, 7 : 7 + CW : C].unsqueeze(2).broadcast_to((P, W, C))
                cs = work.tile([P, CW], f16, tag="cs")
                cs3 = cs[:].rearrange("p (w c) -> p w c", c=C)
                eng = nc.gpsimd if ci % 2 == 0 else nc.vector
                eng.tensor_tensor(cs3, s3, b3, Alu.subtract)

                # square in place on Scalar engine
                nc.scalar.activation(cs[:], cs[:], Act.Square)

                # U[g] = sum_j c[g, j]^2
                cs3 = cs[:].rearrange("p (w c) -> p w c", c=C)
                U = small.tile([P, W], f32, tag="U")
                nc.vector.tensor_reduce(
                    U[:], cs3, axis=mybir.AxisListType.X, op=Alu.add
                )

                # loss[g] = sqrt(U[g] / C); acc[:, ci] = sum_g loss[g]
                lt = small.tile([P, W], f32, tag="loss")
                nc.scalar.activation(
                    lt[:], U[:], Act.Sqrt, scale=1.0 / C,
                    accum_out=acc[:, ci : ci + 1],
                )
            nc.sync.dma_start(o_d[:], acc[:])
    nc.compile()
    return nc


def _make_in_maps(p, q):
    p = np.asarray(p, dtype=np.float32).reshape(B, C).astype(np.float16)
    q = np.asarray(q, dtype=np.float32).reshape(B, C).astype(np.float16)
    in_maps = []
    for i in range(N_CORES):
        in_maps.append(
            {
                "p": np.ascontiguousarray(p[i * BS : (i + 1) * BS]).reshape(P, FPP),
                "q": np.ascontiguousarray(q[i * BS : (i + 1) * BS]).reshape(P, FPP),
            }
        )
    return in_maps


def kernel(p, q, r):
    assert int(r) == 2, f"kernel specialized for r=2, got {r}"
    if "nc" not in _cache:
        _cache["nc"] = _build_program()
    nc = _cache["nc"]

    in_maps = _make_in_maps(p, q)

    from concourse.bass_utils import run_bass_kernel_spmd

    res = run_bass_kernel_spmd(nc, in_maps, list(range(N_CORES)))
    total = 0.0
    for r_ in res.results:
        total += r_["partial"].astype(np.float64).sum()
    return np.float32(total / B)



# revision 2
# speedup vs baseline: 1.4961x; 1.4961x over previous
"""TRN2 Bass/Tile kernel for nn_Loss_58317065945194.

Loss: per-sample EMD with r=2 over C=10 channels:
    d = p - q                       # [B, C]
    S = cumsum(d, axis=1)           # per-sample prefix sums
    per_sample = sqrt(mean(S**2))   # [B]
    out = mean(per_sample)          # scalar

Key idea: the cumsum is linear, so the idle Tensor engine computes it as
S = U^T p - U^T q (U = upper-triangular ones, 10x10) via two PSUM-accumulated
matmuls, fusing the subtract for free. Layout packs 12 samples' channels onto
120 of 128 partitions; each matmul covers 12 x 512 = 6144 samples.

Per 6144-sample tile (per core, 43 tiles):
  - 1 DMA:    x[t] = [128, 1024] fp16 (p cols 0:512 | q cols 512:1024)
  - Tensor:   psum_S  = Wp.T @ p  +  Wq.T @ q      (S, [120+pad, 512] fp32)
  - Scalar:   sq = Square(psum_S) -> SBUF fp16     (psum evacuation + square)
  - Tensor:   psum_U += Wsel[j].T @ sq             (groups of 10 tiles write
              U = sum_c S_c^2 into disjoint 12-row stripes of one psum bank)
  - Scalar:   every 10 tiles: sqrt(U/10) with accum_out -> per-group partial
Host sums the [128, n_groups] partials over cores and divides by B.

Shard: pure data-parallel over B across 8 cores; inputs cast to fp16 and
re-laid-out host-side (zero-padding the 2048 tail samples of each core,
which contribute exactly 0 to the sum).
"""

import sys

import numpy as np

if "/opt/trn_rl_repo" not in sys.path:
    sys.path.insert(0, "/opt/trn_rl_repo")

N_CORES = 8
B, C = 2097152, 10
BS = B // N_CORES        # samples per core shard (262144)
SPB = 12                 # sample-blocks per column (12 * C = 120 rows used)
NW = 512                 # moving free dim / samples per block-row per tile
TPS = SPB * NW           # samples per tile (6144)
NT = -(-BS // TPS)       # tiles per core (43)
SPT = NT * TPS           # padded samples per core (264192)
GRP = 10                 # tiles accumulated per U-psum group (10*12=120 rows)
NG = -(-NT // GRP)       # sqrt groups per core (5)

_cache = {}


def _build_weights():
    """[128, 12, 128] fp16: w[:,0]=Wp, w[:,1]=Wq=-Wp, w[:,2+j]=Wsel_j."""
    w = np.zeros((128, 12, 128), dtype=np.float16)
    for s in range(SPB):
        for c in range(C):
            for i in range(C):
                if c <= i:
                    w[10 * s + c, 0, 10 * s + i] = 1.0
                    w[10 * s + c, 1, 10 * s + i] = -1.0
    for j in range(GRP):
        for s in range(SPB):
            for c in range(C):
                w[10 * s + c, 2 + j, 12 * j + s] = 1.0
    return w


def _build_program():
    import concourse.tile as tile
    from concourse import bacc, mybir

    f32, f16 = mybir.dt.float32, mybir.dt.float16
    Act = mybir.ActivationFunctionType

    nc = bacc.Bacc(
        "TRN2", target_bir_lowering=False, debug=False, num_devices=N_CORES
    )
    x_d = nc.dram_tensor("x", [NT, 128, 2 * NW], f16, kind="ExternalInput").ap()
    w_d = nc.dram_tensor("w", [128, 12, 128], f16, kind="ExternalInput").ap()
    o_d = nc.dram_tensor("partial", [128, NG], f32, kind="ExternalOutput").ap()

    with tile.TileContext(nc) as tc:
        with (
            tc.tile_pool(name="io", bufs=4) as io,
            tc.tile_pool(name="wgt", bufs=1) as wgt,
            tc.tile_pool(name="sqp", bufs=3) as sqp,
            tc.tile_pool(name="junk", bufs=2) as junkp,
            tc.tile_pool(name="accp", bufs=1) as accp,
            tc.tile_pool(name="psS", bufs=4, space="PSUM") as psS,
            tc.tile_pool(name="psU", bufs=2, space="PSUM") as psU,
        ):
            w = wgt.tile([128, 12, 128], f16)
            nc.sync.dma_start(w[:], w_d)
            acc = accp.tile([128, NG], f32)

            psu = None
            pend = None  # (sq, psu, j, t) awaiting the reduce matmul
            for t in range(NT + 1):
                if t < NT:
                    xt = io.tile([128, 2 * NW], f16, tag="x")
                    nc.sync.dma_start(xt[:], x_d[t])
                    ps = psS.tile([128, NW], f32, tag="S")
                    nc.tensor.matmul(
                        ps[:], w[:, 0], xt[:, 0:NW], start=True, stop=False
                    )
                    nc.tensor.matmul(
                        ps[:], w[:, 1], xt[:, NW : 2 * NW], start=False, stop=True
                    )
                    sq = sqp.tile([128, NW], f16, tag="sq")
                    nc.scalar.activation(sq[:], ps[:], Act.Square)
                # reduce matmul for the previous tile (one iteration behind so
                # the Tensor engine never stalls waiting on Scalar's square)
                if pend is not None:
                    psq, ppsu, pj, pt = pend
                    nc.tensor.matmul(
                        ppsu[:],
                        w[:, 2 + pj],
                        psq[:],
                        start=(pj == 0),
                        stop=(pj == GRP - 1 or pt == NT - 1),
                    )
                    if pj == GRP - 1 or pt == NT - 1:
                        g = pt // GRP
                        jk = junkp.tile([128, NW], f16, tag="jk")
                        nc.scalar.activation(
                            jk[:], ppsu[:], Act.Sqrt, scale=1.0 / C,
                            accum_out=acc[:, g : g + 1],
                        )
                if t < NT:
                    j = t % GRP
                    if j == 0:
                        psu = psU.tile([128, NW], f32, tag="U")
                    pend = (sq, psu, j, t)
            nc.sync.dma_start(o_d, acc[:])
    nc.compile()
    return nc


def _make_in_maps(p, q):
    """Lay out each core's shard as [NT, 128, 1024] fp16 tiles.

    Tile t, row 10*s + c, col n      -> p[base + t*6144 + s*512 + n, c]
    Tile t, row 10*s + c, col 512+n  -> q[...same sample..., c]
    Rows 120..127 are zero (matmul weights are zero there too).
    """
    w = _build_weights()

    def lay(a):
        a = np.asarray(a, dtype=np.float32).reshape(B, C).astype(np.float16)
        a = a.reshape(N_CORES, BS, C)
        pad = np.zeros((N_CORES, SPT, C), dtype=np.float16)
        pad[:, :BS] = a
        # [core, t, s, n, c] -> [core, t, s, c, n] -> [core, t, 120, 512]
        v = pad.reshape(N_CORES, NT, SPB, NW, C).transpose(0, 1, 2, 4, 3)
        return np.ascontiguousarray(v).reshape(N_CORES, NT, SPB * C, NW)

    vp, vq = lay(p), lay(q)
    x = np.zeros((N_CORES, NT, 128, 2 * NW), dtype=np.float16)
    x[:, :, : SPB * C, :NW] = vp
    x[:, :, : SPB * C, NW:] = vq
    return [{"x": x[i], "w": w} for i in range(N_CORES)]


def kernel(p, q, r):
    assert int(r) == 2, f"kernel specialized for r=2, got {r}"
    if "nc" not in _cache:
        _cache["nc"] = _build_program()
    nc = _cache["nc"]

    in_maps = _make_in_maps(p, q)

    from concourse.bass_utils import run_bass_kernel_spmd

    res = run_bass_kernel_spmd(nc, in_maps, list(range(N_CORES)))
    total = 0.0
    for r_ in res.results:
        total += r_["partial"].astype(np.float64).sum()
    return np.float32(total / B)
